# revision 8
# baseline (speedup 1.0000x reference)
"""Trainium2 Bass kernel for causal bilinear self-attention (diagonal variant).

Computes, per (b, head):
    scores[t, s] = h[b, t] @ A[head] @ h[b, s]        (causal: s <= t)
    attn = softmax(scores, axis=-1)
    out[b, head, t, :] = attn[t, t] * h[b, t, :]
returned reshaped row-major to (B, T, H*d)  (faithful torch .view semantics).

Only the diagonal of the attention matrix is needed, so the kernel computes
flash-style per-row max / sum-of-exp over the causal triangle plus the
diagonal score, never materializing attention in HBM.

Sharding: 16 (b, head) pairs across 8 cores -> core c handles b = c // 4,
heads 2*(c%4) and 2*(c%4)+1.  Each core receives h[b] (4 MB) and its two
A matrices; outputs its two (T, d) slices.

Precision: the TensorE fp32 path costs 4 cyc/row; float32r (TF32-like,
11-bit mantissa, exact MAC on rounded inputs) costs 1 cyc/row at N>=256.
Splitting an fp32 value x into xh = round_f32r(x), xl = x - xh (both exactly
representable in f32r) lets multi-pass f32r matmuls recover near-fp32
precision at 2-3 cyc/row.  STAGE1/STAGE2 below select the scheme per stage.

Hardware notes (found empirically on this axon/neuronxcc toolchain):
  - tensor_tensor_reduce with a PSUM input crashes the device; so does an
    ACT read of a PSUM region modified in place by the DVE.  PSUM is
    therefore written only by the PE and read only by DVE copy-class ops;
    all masking / softmax arithmetic happens on SBUF copies.
  - mask constants are DMA'd from host inputs (no gpsimd affine_select).
"""

import os
import sys

try:
    import concourse.bass  # noqa: F401
except ImportError:  # pragma: no cover
    sys.path.insert(0, "/opt/trn_rl_repo")

import numpy as np

import concourse.bass as bass  # noqa: F401
import concourse.tile as tile
from concourse import bacc, bass_utils, mybir

B, T, D, H = 2, 2048, 512, 8
NCORES = 8
P = 128
NT = T // P      # 16 row tiles
ND = D // P      # 4 contraction chunks
SCH = 512        # s-chunk width (one PSUM bank of fp32)
NEG = -1.0e30

f32 = mybir.dt.float32
f32r = mybir.dt.float32r

# stage1 (g = h @ A): "f32" exact 4 cyc/row | "r3" ~exact 3 | "r2" 2 | "r1" 1
# stage2 (scores = g @ h^T): "f32" 4 | "r3" ~exact 3 | "r2" 2 | "r1" 1
STAGE1 = os.environ.get("BK_STAGE1", "f32")
STAGE2 = os.environ.get("BK_STAGE2", "r2")


def build_nc(stage1=None, stage2=None):
    stage1 = stage1 or STAGE1
    stage2 = stage2 or STAGE2
    assert stage1 in ("f32", "r3", "r2", "r1") and stage2 in ("f32", "r3", "r2", "r1")
    s2_r = stage2 != "f32"
    need_hT32 = stage1 == "f32" or stage2 == "f32"
    need_hTr = stage1 != "f32" or stage2 != "f32"
    need_hTl = stage1 == "r3" or stage2 == "r3"
    need_Ar = stage1 != "f32"
    need_Al = stage1 in ("r3", "r2")

    nc = bacc.Bacc("TRN2", target_bir_lowering=False, debug=False)
    hb = nc.dram_tensor("hb", [T, D], f32, kind="ExternalInput")
    A2 = nc.dram_tensor("A2", [2, D, D], f32, kind="ExternalInput")
    cmaskd = nc.dram_tensor("cmaskd", [P, P], f32, kind="ExternalInput")
    identd = nc.dram_tensor("identd", [P, P], f32, kind="ExternalInput")
    out2 = nc.dram_tensor("out2", [2, T, D], f32, kind="ExternalOutput")
    hb_t = hb[:].rearrange("(n p) d -> p n d", p=P)  # [128, 16, 512] view

    with tile.TileContext(nc) as tc:
        with (
            tc.tile_pool(name="const", bufs=1) as constp,
            tc.tile_pool(name="big", bufs=1) as big,
            tc.tile_pool(name="gpool", bufs=1) as gpool,
            tc.tile_pool(name="hin", bufs=3) as hin,
            tc.tile_pool(name="psum", bufs=8, space="PSUM") as psum,
            tc.tile_pool(name="scs", bufs=3) as scs,
            tc.tile_pool(name="escr", bufs=2) as escr,
            tc.tile_pool(name="stats", bufs=4) as stats,
            tc.tile_pool(name="outp", bufs=3) as outp,
        ):
            ident = constp.tile([P, P], f32)
            nc.sync.dma_start(out=ident, in_=identd[:])
            cmask = constp.tile([P, P], f32)
            nc.sync.dma_start(out=cmask, in_=cmaskd[:])

            # A: A_sb[p, hd, c, e] = A[hd, c*128 + p, e]
            A_sb = big.tile([P, 2, ND, D], f32)
            nc.sync.dma_start(
                out=A_sb, in_=A2[:].rearrange("h (c p) e -> p h c e", p=P)
            )
            if need_Ar:
                A_r = big.tile([P, 2, ND, D], f32r)
                nc.vector.tensor_copy(A_r, A_sb)
            if need_Al:
                A_l = big.tile([P, 2, ND, D], f32r)
                nc.vector.tensor_sub(A_l, A_sb, A_r.bitcast(f32))

            # h^T: hT*[p, c, t] = h[t, c*128 + p], via PE transpose
            hT32 = big.tile([P, ND, T], f32, name="hT32") if need_hT32 else None
            hTr = big.tile([P, ND, T], f32r, name="hTr") if need_hTr else None
            hTl = big.tile([P, ND, T], f32r, name="hTl") if need_hTl else None
            for i in range(NT):
                hrow = hin.tile([P, D], f32, tag="hrow")
                nc.sync.dma_start(out=hrow, in_=hb_t[:, i, :])
                for c in range(ND):
                    pt = psum.tile([P, P], f32, tag="ps")
                    nc.tensor.transpose(pt, hrow[:, c * P : (c + 1) * P], ident)
                    tsl = slice(i * P, (i + 1) * P)
                    if need_hT32:
                        nc.vector.tensor_copy(hT32[:, c, tsl], pt)
                    if need_hTr:
                        nc.vector.tensor_copy(hTr[:, c, tsl], pt)
                    if need_hTl:
                        nc.vector.tensor_sub(
                            hTl[:, c, tsl], pt, hTr[:, c, tsl].bitcast(f32)
                        )

            for hd in range(2):
                # ---- stage 1: gT[e, t] = sum_d A[d, e] * hT[d, t] ----
                need_g32 = stage2 == "f32"
                need_gh = s2_r
                need_gl = stage2 in ("r3", "r2")
                gT32 = gpool.tile([P, ND, T], f32, tag="g32", name="gT32") if need_g32 else None
                gTh = gpool.tile([P, ND, T], f32r, tag="gh", name="gTh") if need_gh else None
                gTl = gpool.tile([P, ND, T], f32r, tag="gl", name="gTl") if need_gl else None

                if stage1 == "f32":
                    s1_passes = [(A_sb, hT32)]
                elif stage1 == "r1":
                    s1_passes = [(A_r, hTr)]
                elif stage1 == "r2":
                    s1_passes = [(A_r, hTr), (A_l, hTr)]
                else:  # r3
                    s1_passes = [(A_r, hTr), (A_l, hTr), (A_r, hTl)]

                for ec in range(ND):
                    ecs = slice(ec * P, (ec + 1) * P)
                    for tsl in range(T // SCH):
                        ts_ = slice(tsl * SCH, (tsl + 1) * SCH)
                        pg = psum.tile([P, SCH], f32, tag="ps")
                        nmm = len(s1_passes) * ND
                        k = 0
                        for lhs_src, rhs_src in s1_passes:
                            for dc in range(ND):
                                nc.tensor.matmul(
                                    pg,
                                    lhs_src[:, hd, dc, ecs],
                                    rhs_src[:, dc, ts_],
                                    start=(k == 0),
                                    stop=(k == nmm - 1),
                                )
                                k += 1
                        if gT32 is not None:
                            nc.vector.tensor_copy(gT32[:, ec, ts_], pg)
                        if gTh is not None:
                            nc.vector.tensor_copy(gTh[:, ec, ts_], pg)
                        if gTl is not None:
                            nc.vector.tensor_sub(
                                gTl[:, ec, ts_], pg, gTh[:, ec, ts_].bitcast(f32)
                            )

                if stage2 == "f32":
                    s2_passes = [(gT32, hT32)]
                elif stage2 == "r3":
                    s2_passes = [(gTh, hTr), (gTl, hTr), (gTh, hTl)]
                elif stage2 == "r2":
                    s2_passes = [(gTh, hTr), (gTl, hTr)]
                else:
                    s2_passes = [(gTh, hTr)]

                # ---- stage 2 + softmax diag, per row tile ----
                for i in range(NT):
                    nch = i // 4 + 1
                    its = slice(i * P, (i + 1) * P)
                    dcol = (i % 4) * P       # diag block start within last chunk
                    wlast = (i % 4 + 1) * P  # causal width of last chunk
                    # f32r matmuls need moving dim >= 256 for full rate; widen
                    # the 128-wide matmul (extra cols never copied out of PSUM)
                    w_mm = max(wlast, 2 * P) if s2_r else wlast

                    m4 = stats.tile([P, 4], f32, tag="m4")
                    lp = stats.tile([P, 4], f32, tag="lp")
                    chunks = []
                    for j in range(nch):
                        last = j == nch - 1
                        w = w_mm if last else SCH
                        wc = wlast if last else SCH  # causal (copied) width
                        ps = psum.tile([P, SCH], f32, tag="ps")
                        nmm = len(s2_passes) * ND
                        k = 0
                        for lhs_src, rhs_src in s2_passes:
                            for ec in range(ND):
                                nc.tensor.matmul(
                                    ps[:, :w],
                                    lhs_src[:, ec, its],
                                    rhs_src[:, ec, j * SCH : j * SCH + w],
                                    start=(k == 0),
                                    stop=(k == nmm - 1),
                                )
                                k += 1
                        if last:
                            # diag chunk: SBUF copy + causal mask (PSUM must
                            # stay PE-written-only for ACT readers)
                            sc = scs.tile([P, SCH], f32, tag="sc")
                            nc.vector.tensor_copy(sc[:, :wc], ps[:, :wc])
                            nc.vector.tensor_add(
                                sc[:, dcol : dcol + P], sc[:, dcol : dcol + P], cmask
                            )
                            src_t = sc
                        else:
                            src_t = ps
                        nc.vector.reduce_max(
                            out=m4[:, j : j + 1], in_=src_t[:, :wc],
                            axis=mybir.AxisListType.X,
                        )
                        chunks.append((src_t, wc))

                    nm = stats.tile([P, 1], f32, tag="nm")
                    nc.vector.reduce_max(
                        out=nm, in_=m4[:, :nch], axis=mybir.AxisListType.X, negate=True
                    )
                    ex_last = None
                    for j, (sc, wc) in enumerate(chunks):
                        ex = escr.tile([P, SCH], f32, tag="ex")
                        nc.scalar.activation(
                            out=ex[:, :wc],
                            in_=sc[:, :wc],
                            func=mybir.ActivationFunctionType.Exp,
                            bias=nm,
                            scale=1.0,
                            accum_out=lp[:, j : j + 1],
                        )
                        if j == nch - 1:
                            ex_last = ex
                    # diag of exp block: mul by identity then row-reduce
                    dscr = stats.tile([P, P], f32, tag="dscr")
                    nc.vector.tensor_mul(dscr, ex_last[:, dcol : dcol + P], ident)
                    ediag = stats.tile([P, 1], f32, tag="ediag")
                    nc.vector.reduce_sum(
                        out=ediag, in_=dscr, axis=mybir.AxisListType.X
                    )
                    lsum = stats.tile([P, 1], f32, tag="lsum")
                    nc.vector.reduce_sum(
                        out=lsum, in_=lp[:, :nch], axis=mybir.AxisListType.X
                    )
                    rl = stats.tile([P, 1], f32, tag="rl")
                    nc.vector.reciprocal(rl, lsum)
                    datt = stats.tile([P, 1], f32, tag="datt")
                    nc.vector.tensor_mul(datt, ediag, rl)

                    hrow2 = hin.tile([P, D], f32, tag="hrow2")
                    nc.sync.dma_start(out=hrow2, in_=hb_t[:, i, :])
                    ot = outp.tile([P, D], f32, tag="ot")
                    nc.vector.tensor_scalar_mul(ot, hrow2, datt)
                    nc.sync.dma_start(out=out2[hd, its, :], in_=ot)

    nc.compile()
    return nc


_NC_CACHE = {}


def _get_nc(stage1=None, stage2=None):
    key = (stage1 or STAGE1, stage2 or STAGE2)
    if key not in _NC_CACHE:
        _NC_CACHE[key] = build_nc(*key)
    return _NC_CACHE[key]


def _consts():
    cmask = np.triu(np.full((P, P), NEG, np.float32), 1)
    ident = np.eye(P, dtype=np.float32)
    return cmask, ident


def make_in_maps(h, A):
    h = np.ascontiguousarray(h, dtype=np.float32)
    A = np.ascontiguousarray(A, dtype=np.float32)
    cmask, ident = _consts()
    in_maps = []
    for c in range(NCORES):
        b = c // 4
        h0 = 2 * (c % 4)
        in_maps.append({"hb": h[b], "A2": np.ascontiguousarray(A[h0 : h0 + 2]),
                        "cmaskd": cmask, "identd": ident})
    return in_maps


def assemble(results):
    full = np.empty((B, H, T, D), dtype=np.float32)
    for c in range(NCORES):
        b = c // 4
        h0 = 2 * (c % 4)
        o = results[c]["out2"]
        full[b, h0] = o[0]
        full[b, h0 + 1] = o[1]
    return full.reshape(B, T, H * D)


def kernel(h, A):
    nc = _get_nc()
    res = bass_utils.run_bass_kernel_spmd(
        nc, make_in_maps(h, A), core_ids=list(range(NCORES))
    )
    return assemble(res.results)


# revision 10
# speedup vs baseline: 1.1847x; 1.1847x over previous
"""Trainium2 Bass kernel for causal bilinear self-attention (diagonal variant).

Computes, per (b, head):
    scores[t, s] = h[b, t] @ A[head] @ h[b, s]        (causal: s <= t)
    attn = softmax(scores, axis=-1)
    out[b, head, t, :] = attn[t, t] * h[b, t, :]
returned reshaped row-major to (B, T, H*d)  (faithful torch .view semantics).

Only the diagonal of the attention matrix is needed, so the kernel computes
flash-style per-row max / sum-of-exp over the causal triangle plus the
diagonal score, never materializing attention in HBM.

Sharding: 16 (b, head) pairs across 8 cores -> core c handles b = c // 4,
heads 2*(c%4) and 2*(c%4)+1.  Each core receives h[b] (4 MB) and its two
A matrices; outputs its two (T, d) slices.

Precision: the TensorE fp32 path costs 4 cyc/row; float32r (TF32-like,
11-bit mantissa, exact MAC on rounded inputs) costs 1 cyc/row at N>=256.
Splitting an fp32 value x into xh = round_f32r(x), xl = x - xh (both exactly
representable in f32r) lets multi-pass f32r matmuls recover near-fp32
precision at 2-3 cyc/row.  STAGE1/STAGE2 below select the scheme per stage.

Hardware notes (found empirically on this axon/neuronxcc toolchain):
  - tensor_tensor_reduce with a PSUM input crashes the device; so does an
    ACT read of a PSUM region modified in place by the DVE.  PSUM is
    therefore written only by the PE and read only by DVE copy-class ops;
    all masking / softmax arithmetic happens on SBUF copies.
  - mask constants are DMA'd from host inputs (no gpsimd affine_select).
"""

import os
import sys

try:
    import concourse.bass  # noqa: F401
except ImportError:  # pragma: no cover
    sys.path.insert(0, "/opt/trn_rl_repo")

import numpy as np

import concourse.bass as bass  # noqa: F401
import concourse.tile as tile
from concourse import bacc, bass_utils, mybir

B, T, D, H = 2, 2048, 512, 8
NCORES = 8
P = 128
NT = T // P      # 16 row tiles
ND = D // P      # 4 contraction chunks
SCH = 512        # s-chunk width (one PSUM bank of fp32)
NEG = -1.0e30

f32 = mybir.dt.float32
f32r = mybir.dt.float32r

# stage1 (g = h @ A): "f32" exact 4 cyc/row | "r3" ~exact 3 | "r2" 2 | "r1" 1
# stage2 (scores = g @ h^T): "f32" 4 | "r3" ~exact 3 | "r2" 2 | "r1" 1
# Default r3/r3: 3-pass f32r split per stage -> ~fp32 accuracy (measured
# 2.3e-6 rel-to-max vs fp64 reference) at ~317 us/core predicted, vs 381 us
# for the plain fp32 path (0.0 measured error) and 154 us for r1/r1 (1.4e-3).
STAGE1 = os.environ.get("BK_STAGE1", "r3")
STAGE2 = os.environ.get("BK_STAGE2", "r3")


def build_nc(stage1=None, stage2=None):
    stage1 = stage1 or STAGE1
    stage2 = stage2 or STAGE2
    assert stage1 in ("f32", "r3", "r2", "r1") and stage2 in ("f32", "r3", "r2", "r1")
    s2_r = stage2 != "f32"
    need_hT32 = stage1 == "f32" or stage2 == "f32"
    need_hTr = stage1 != "f32" or stage2 != "f32"
    need_hTl = stage1 == "r3" or stage2 == "r3"
    need_Ar = stage1 != "f32"
    need_Al = stage1 in ("r3", "r2")

    nc = bacc.Bacc("TRN2", target_bir_lowering=False, debug=False)
    hb = nc.dram_tensor("hb", [T, D], f32, kind="ExternalInput")
    A2 = nc.dram_tensor("A2", [2, D, D], f32, kind="ExternalInput")
    cmaskd = nc.dram_tensor("cmaskd", [P, P], f32, kind="ExternalInput")
    identd = nc.dram_tensor("identd", [P, P], f32, kind="ExternalInput")
    out2 = nc.dram_tensor("out2", [2, T, D], f32, kind="ExternalOutput")
    hb_t = hb[:].rearrange("(n p) d -> p n d", p=P)  # [128, 16, 512] view

    with tile.TileContext(nc) as tc:
        with (
            tc.tile_pool(name="const", bufs=1) as constp,
            tc.tile_pool(name="big", bufs=1) as big,
            tc.tile_pool(name="gpool", bufs=1) as gpool,
            tc.tile_pool(name="hin", bufs=3) as hin,
            tc.tile_pool(name="psum", bufs=8, space="PSUM") as psum,
            tc.tile_pool(name="scs", bufs=3) as scs,
            tc.tile_pool(name="escr", bufs=2) as escr,
            tc.tile_pool(name="stats", bufs=4) as stats,
            tc.tile_pool(name="outp", bufs=2) as outp,
        ):
            ident = constp.tile([P, P], f32)
            nc.sync.dma_start(out=ident, in_=identd[:])
            cmask = constp.tile([P, P], f32)
            nc.sync.dma_start(out=cmask, in_=cmaskd[:])

            # A: A_sb[p, hd, c, e] = A[hd, c*128 + p, e]
            A_sb = big.tile([P, 2, ND, D], f32)
            nc.sync.dma_start(
                out=A_sb, in_=A2[:].rearrange("h (c p) e -> p h c e", p=P)
            )
            if need_Ar:
                A_r = big.tile([P, 2, ND, D], f32r)
                nc.vector.tensor_copy(A_r, A_sb)
            if need_Al:
                A_l = big.tile([P, 2, ND, D], f32r)
                nc.vector.tensor_sub(A_l, A_sb, A_r.bitcast(f32))

            # h^T: hT*[p, c, t] = h[t, c*128 + p], via PE transpose
            hT32 = big.tile([P, ND, T], f32, name="hT32") if need_hT32 else None
            hTr = big.tile([P, ND, T], f32r, name="hTr") if need_hTr else None
            hTl = big.tile([P, ND, T], f32r, name="hTl") if need_hTl else None
            for i in range(NT):
                hrow = hin.tile([P, D], f32, tag="hrow")
                nc.sync.dma_start(out=hrow, in_=hb_t[:, i, :])
                for c in range(ND):
                    pt = psum.tile([P, P], f32, tag="ps")
                    nc.tensor.transpose(pt, hrow[:, c * P : (c + 1) * P], ident)
                    tsl = slice(i * P, (i + 1) * P)
                    if need_hT32:
                        nc.vector.tensor_copy(hT32[:, c, tsl], pt)
                    if need_hTr:
                        nc.vector.tensor_copy(hTr[:, c, tsl], pt)
                    if need_hTl:
                        nc.vector.tensor_sub(
                            hTl[:, c, tsl], pt, hTr[:, c, tsl].bitcast(f32)
                        )

            for hd in range(2):
                # ---- stage 1: gT[e, t] = sum_d A[d, e] * hT[d, t] ----
                need_g32 = stage2 == "f32"
                need_gh = s2_r
                need_gl = stage2 in ("r3", "r2")
                gT32 = gpool.tile([P, ND, T], f32, tag="g32", name="gT32") if need_g32 else None
                gTh = gpool.tile([P, ND, T], f32r, tag="gh", name="gTh") if need_gh else None
                gTl = gpool.tile([P, ND, T], f32r, tag="gl", name="gTl") if need_gl else None

                if stage1 == "f32":
                    s1_passes = [(A_sb, hT32)]
                elif stage1 == "r1":
                    s1_passes = [(A_r, hTr)]
                elif stage1 == "r2":
                    s1_passes = [(A_r, hTr), (A_l, hTr)]
                else:  # r3
                    s1_passes = [(A_r, hTr), (A_l, hTr), (A_r, hTl)]

                for ec in range(ND):
                    ecs = slice(ec * P, (ec + 1) * P)
                    for tsl in range(T // SCH):
                        ts_ = slice(tsl * SCH, (tsl + 1) * SCH)
                        pg = psum.tile([P, SCH], f32, tag="ps")
                        nmm = len(s1_passes) * ND
                        k = 0
                        for lhs_src, rhs_src in s1_passes:
                            for dc in range(ND):
                                nc.tensor.matmul(
                                    pg,
                                    lhs_src[:, hd, dc, ecs],
                                    rhs_src[:, dc, ts_],
                                    start=(k == 0),
                                    stop=(k == nmm - 1),
                                )
                                k += 1
                        if gT32 is not None:
                            nc.vector.tensor_copy(gT32[:, ec, ts_], pg)
                        if gTh is not None:
                            nc.vector.tensor_copy(gTh[:, ec, ts_], pg)
                        if gTl is not None:
                            nc.vector.tensor_sub(
                                gTl[:, ec, ts_], pg, gTh[:, ec, ts_].bitcast(f32)
                            )

                if stage2 == "f32":
                    s2_passes = [(gT32, hT32)]
                elif stage2 == "r3":
                    s2_passes = [(gTh, hTr), (gTl, hTr), (gTh, hTl)]
                elif stage2 == "r2":
                    s2_passes = [(gTh, hTr), (gTl, hTr)]
                else:
                    s2_passes = [(gTh, hTr)]

                # ---- stage 2 + softmax diag, per row tile ----
                for i in range(NT):
                    nch = i // 4 + 1
                    its = slice(i * P, (i + 1) * P)
                    dcol = (i % 4) * P       # diag block start within last chunk
                    wlast = (i % 4 + 1) * P  # causal width of last chunk
                    # f32r matmuls need moving dim >= 256 for full rate; widen
                    # the 128-wide matmul (extra cols never copied out of PSUM)
                    w_mm = max(wlast, 2 * P) if s2_r else wlast

                    m4 = stats.tile([P, 4], f32, tag="m4")
                    lp = stats.tile([P, 4], f32, tag="lp")
                    chunks = []
                    for j in range(nch):
                        last = j == nch - 1
                        w = w_mm if last else SCH
                        wc = wlast if last else SCH  # causal (copied) width
                        ps = psum.tile([P, SCH], f32, tag="ps")
                        nmm = len(s2_passes) * ND
                        k = 0
                        for lhs_src, rhs_src in s2_passes:
                            for ec in range(ND):
                                nc.tensor.matmul(
                                    ps[:, :w],
                                    lhs_src[:, ec, its],
                                    rhs_src[:, ec, j * SCH : j * SCH + w],
                                    start=(k == 0),
                                    stop=(k == nmm - 1),
                                )
                                k += 1
                        if last:
                            # diag chunk: SBUF copy + causal mask (PSUM must
                            # stay PE-written-only for ACT readers)
                            sc = scs.tile([P, SCH], f32, tag="sc")
                            nc.vector.tensor_copy(sc[:, :wc], ps[:, :wc])
                            nc.vector.tensor_add(
                                sc[:, dcol : dcol + P], sc[:, dcol : dcol + P], cmask
                            )
                            src_t = sc
                        else:
                            src_t = ps
                        nc.vector.reduce_max(
                            out=m4[:, j : j + 1], in_=src_t[:, :wc],
                            axis=mybir.AxisListType.X,
                        )
                        chunks.append((src_t, wc))

                    nm = stats.tile([P, 1], f32, tag="nm")
                    nc.vector.reduce_max(
                        out=nm, in_=m4[:, :nch], axis=mybir.AxisListType.X, negate=True
                    )
                    ex_last = None
                    for j, (sc, wc) in enumerate(chunks):
                        ex = escr.tile([P, SCH], f32, tag="ex")
                        nc.scalar.activation(
                            out=ex[:, :wc],
                            in_=sc[:, :wc],
                            func=mybir.ActivationFunctionType.Exp,
                            bias=nm,
                            scale=1.0,
                            accum_out=lp[:, j : j + 1],
                        )
                        if j == nch - 1:
                            ex_last = ex
                    # diag of exp block: mul by identity then row-reduce
                    dscr = stats.tile([P, P], f32, tag="dscr")
                    nc.vector.tensor_mul(dscr, ex_last[:, dcol : dcol + P], ident)
                    ediag = stats.tile([P, 1], f32, tag="ediag")
                    nc.vector.reduce_sum(
                        out=ediag, in_=dscr, axis=mybir.AxisListType.X
                    )
                    lsum = stats.tile([P, 1], f32, tag="lsum")
                    nc.vector.reduce_sum(
                        out=lsum, in_=lp[:, :nch], axis=mybir.AxisListType.X
                    )
                    rl = stats.tile([P, 1], f32, tag="rl")
                    nc.vector.reciprocal(rl, lsum)
                    datt = stats.tile([P, 1], f32, tag="datt")
                    nc.vector.tensor_mul(datt, ediag, rl)

                    hrow2 = hin.tile([P, D], f32, tag="hrow2")
                    nc.sync.dma_start(out=hrow2, in_=hb_t[:, i, :])
                    ot = outp.tile([P, D], f32, tag="ot")
                    nc.vector.tensor_scalar_mul(ot, hrow2, datt)
                    nc.sync.dma_start(out=out2[hd, its, :], in_=ot)

    nc.compile()
    return nc


_NC_CACHE = {}


def _get_nc(stage1=None, stage2=None):
    key = (stage1 or STAGE1, stage2 or STAGE2)
    if key not in _NC_CACHE:
        _NC_CACHE[key] = build_nc(*key)
    return _NC_CACHE[key]


def _consts():
    cmask = np.triu(np.full((P, P), NEG, np.float32), 1)
    ident = np.eye(P, dtype=np.float32)
    return cmask, ident


def make_in_maps(h, A):
    h = np.ascontiguousarray(h, dtype=np.float32)
    A = np.ascontiguousarray(A, dtype=np.float32)
    cmask, ident = _consts()
    in_maps = []
    for c in range(NCORES):
        b = c // 4
        h0 = 2 * (c % 4)
        in_maps.append({"hb": h[b], "A2": np.ascontiguousarray(A[h0 : h0 + 2]),
                        "cmaskd": cmask, "identd": ident})
    return in_maps


def assemble(results):
    full = np.empty((B, H, T, D), dtype=np.float32)
    for c in range(NCORES):
        b = c // 4
        h0 = 2 * (c % 4)
        o = results[c]["out2"]
        full[b, h0] = o[0]
        full[b, h0 + 1] = o[1]
    return full.reshape(B, T, H * D)


def kernel(h, A):
    nc = _get_nc()
    res = bass_utils.run_bass_kernel_spmd(
        nc, make_in_maps(h, A), core_ids=list(range(NCORES))
    )
    return assemble(res.results)


# revision 11
# speedup vs baseline: 1.1941x; 1.0079x over previous
"""Trainium2 Bass kernel for causal bilinear self-attention (diagonal variant).

Computes, per (b, head):
    scores[t, s] = h[b, t] @ A[head] @ h[b, s]        (causal: s <= t)
    attn = softmax(scores, axis=-1)
    out[b, head, t, :] = attn[t, t] * h[b, t, :]
returned reshaped row-major to (B, T, H*d)  (faithful torch .view semantics).

Only the diagonal of the attention matrix is needed, so the kernel computes
flash-style per-row max / sum-of-exp over the causal triangle plus the
diagonal score, never materializing attention in HBM.

Sharding: 16 (b, head) pairs across 8 cores -> core c handles b = c // 4,
heads 2*(c%4) and 2*(c%4)+1.  Each core receives h[b] (4 MB) and its two
A matrices; outputs its two (T, d) slices.

Precision: the TensorE fp32 path costs 4 cyc/row; float32r (TF32-like,
11-bit mantissa, exact MAC on rounded inputs) costs 1 cyc/row at N>=256.
Splitting an fp32 value x into xh = round_f32r(x), xl = x - xh (both exactly
representable in f32r) lets multi-pass f32r matmuls recover near-fp32
precision at 2-3 cyc/row.  STAGE1/STAGE2 below select the scheme per stage.

Hardware notes (found empirically on this axon/neuronxcc toolchain):
  - tensor_tensor_reduce with a PSUM input crashes the device; so does an
    ACT read of a PSUM region modified in place by the DVE.  PSUM is
    therefore written only by the PE and read only by DVE copy-class ops;
    all masking / softmax arithmetic happens on SBUF copies.
  - mask constants are DMA'd from host inputs (no gpsimd affine_select).
"""

import os
import sys

try:
    import concourse.bass  # noqa: F401
except ImportError:  # pragma: no cover
    sys.path.insert(0, "/opt/trn_rl_repo")

import numpy as np

import concourse.bass as bass  # noqa: F401
import concourse.tile as tile
from concourse import bacc, bass_utils, mybir

B, T, D, H = 2, 2048, 512, 8
NCORES = 8
P = 128
NT = T // P      # 16 row tiles
ND = D // P      # 4 contraction chunks
SCH = 512        # s-chunk width (one PSUM bank of fp32)
NEG = -1.0e30

f32 = mybir.dt.float32
f32r = mybir.dt.float32r

# stage1 (g = h @ A): "f32" exact 4 cyc/row | "r3" ~exact 3 | "r2" 2 | "r1" 1
# stage2 (scores = g @ h^T): "f32" 4 | "r3" ~exact 3 | "r2" 2 | "r1" 1
# Default r3/r3: 3-pass f32r split per stage -> ~fp32 accuracy (measured
# 2.3e-6 rel-to-max vs fp64 reference) at ~317 us/core predicted, vs 381 us
# for the plain fp32 path (0.0 measured error) and 154 us for r1/r1 (1.4e-3).
STAGE1 = os.environ.get("BK_STAGE1", "r3")
STAGE2 = os.environ.get("BK_STAGE2", "r3")


def build_nc(stage1=None, stage2=None):
    stage1 = stage1 or STAGE1
    stage2 = stage2 or STAGE2
    assert stage1 in ("f32", "r3", "r2", "r1") and stage2 in ("f32", "r3", "r2", "r1")
    s2_r = stage2 != "f32"
    need_hT32 = stage1 == "f32" or stage2 == "f32"
    need_hTr = stage1 != "f32" or stage2 != "f32"
    need_hTl = stage1 == "r3" or stage2 == "r3"
    need_Ar = stage1 != "f32"
    need_Al = stage1 in ("r3", "r2")

    nc = bacc.Bacc("TRN2", target_bir_lowering=False, debug=False)
    hb = nc.dram_tensor("hb", [T, D], f32, kind="ExternalInput")
    A2 = nc.dram_tensor("A2", [2, D, D], f32, kind="ExternalInput")
    cmaskd = nc.dram_tensor("cmaskd", [P, P], f32, kind="ExternalInput")
    identd = nc.dram_tensor("identd", [P, P], f32, kind="ExternalInput")
    out2 = nc.dram_tensor("out2", [2, T, D], f32, kind="ExternalOutput")
    hb_t = hb[:].rearrange("(n p) d -> p n d", p=P)  # [128, 16, 512] view

    with tile.TileContext(nc) as tc:
        with (
            tc.tile_pool(name="const", bufs=1) as constp,
            tc.tile_pool(name="big", bufs=1) as big,
            tc.tile_pool(name="gpool", bufs=1) as gpool,
            tc.tile_pool(name="hin", bufs=3) as hin,
            tc.tile_pool(name="psum", bufs=8, space="PSUM") as psum,
            tc.tile_pool(name="scs", bufs=3) as scs,
            tc.tile_pool(name="escr", bufs=2) as escr,
            tc.tile_pool(name="stats", bufs=4) as stats,
            tc.tile_pool(name="outp", bufs=2) as outp,
        ):
            ident = constp.tile([P, P], f32)
            nc.gpsimd.dma_start(out=ident, in_=identd[:])
            cmask = constp.tile([P, P], f32)
            nc.gpsimd.dma_start(out=cmask, in_=cmaskd[:])

            # A: A_sb[p, hd, c, e] = A[hd, c*128 + p, e]
            A_sb = big.tile([P, 2, ND, D], f32)
            nc.gpsimd.dma_start(
                out=A_sb, in_=A2[:].rearrange("h (c p) e -> p h c e", p=P)
            )
            if need_Ar:
                A_r = big.tile([P, 2, ND, D], f32r)
                nc.vector.tensor_copy(A_r, A_sb)
            if need_Al:
                A_l = big.tile([P, 2, ND, D], f32r)
                nc.vector.tensor_sub(A_l, A_sb, A_r.bitcast(f32))

            # h^T: hT*[p, c, t] = h[t, c*128 + p], via PE transpose
            hT32 = big.tile([P, ND, T], f32, name="hT32") if need_hT32 else None
            hTr = big.tile([P, ND, T], f32r, name="hTr") if need_hTr else None
            hTl = big.tile([P, ND, T], f32r, name="hTl") if need_hTl else None
            for i in range(NT):
                hrow = hin.tile([P, D], f32, tag="hrow")
                nc.sync.dma_start(out=hrow, in_=hb_t[:, i, :])
                for c in range(ND):
                    pt = psum.tile([P, P], f32, tag="ps")
                    nc.tensor.transpose(pt, hrow[:, c * P : (c + 1) * P], ident)
                    tsl = slice(i * P, (i + 1) * P)
                    if need_hT32:
                        nc.vector.tensor_copy(hT32[:, c, tsl], pt)
                    if need_hTr:
                        nc.vector.tensor_copy(hTr[:, c, tsl], pt)
                    if need_hTl:
                        nc.vector.tensor_sub(
                            hTl[:, c, tsl], pt, hTr[:, c, tsl].bitcast(f32)
                        )

            for hd in range(2):
                # ---- stage 1: gT[e, t] = sum_d A[d, e] * hT[d, t] ----
                need_g32 = stage2 == "f32"
                need_gh = s2_r
                need_gl = stage2 in ("r3", "r2")
                gT32 = gpool.tile([P, ND, T], f32, tag="g32", name="gT32") if need_g32 else None
                gTh = gpool.tile([P, ND, T], f32r, tag="gh", name="gTh") if need_gh else None
                gTl = gpool.tile([P, ND, T], f32r, tag="gl", name="gTl") if need_gl else None

                if stage1 == "f32":
                    s1_passes = [(A_sb, hT32)]
                elif stage1 == "r1":
                    s1_passes = [(A_r, hTr)]
                elif stage1 == "r2":
                    s1_passes = [(A_r, hTr), (A_l, hTr)]
                else:  # r3
                    s1_passes = [(A_r, hTr), (A_l, hTr), (A_r, hTl)]

                for ec in range(ND):
                    ecs = slice(ec * P, (ec + 1) * P)
                    for tsl in range(T // SCH):
                        ts_ = slice(tsl * SCH, (tsl + 1) * SCH)
                        pg = psum.tile([P, SCH], f32, tag="ps")
                        nmm = len(s1_passes) * ND
                        k = 0
                        for lhs_src, rhs_src in s1_passes:
                            for dc in range(ND):
                                nc.tensor.matmul(
                                    pg,
                                    lhs_src[:, hd, dc, ecs],
                                    rhs_src[:, dc, ts_],
                                    start=(k == 0),
                                    stop=(k == nmm - 1),
                                )
                                k += 1
                        if gT32 is not None:
                            nc.vector.tensor_copy(gT32[:, ec, ts_], pg)
                        if gTh is not None:
                            nc.vector.tensor_copy(gTh[:, ec, ts_], pg)
                        if gTl is not None:
                            nc.vector.tensor_sub(
                                gTl[:, ec, ts_], pg, gTh[:, ec, ts_].bitcast(f32)
                            )

                if stage2 == "f32":
                    s2_passes = [(gT32, hT32)]
                elif stage2 == "r3":
                    s2_passes = [(gTh, hTr), (gTl, hTr), (gTh, hTl)]
                elif stage2 == "r2":
                    s2_passes = [(gTh, hTr), (gTl, hTr)]
                else:
                    s2_passes = [(gTh, hTr)]

                # ---- stage 2 + softmax diag, per row tile ----
                for i in range(NT):
                    nch = i // 4 + 1
                    its = slice(i * P, (i + 1) * P)
                    dcol = (i % 4) * P       # diag block start within last chunk
                    wlast = (i % 4 + 1) * P  # causal width of last chunk
                    # f32r matmuls need moving dim >= 256 for full rate; widen
                    # the 128-wide matmul (extra cols never copied out of PSUM)
                    w_mm = max(wlast, 2 * P) if s2_r else wlast

                    m4 = stats.tile([P, 4], f32, tag="m4")
                    lp = stats.tile([P, 4], f32, tag="lp")
                    chunks = []
                    for j in range(nch):
                        last = j == nch - 1
                        w = w_mm if last else SCH
                        wc = wlast if last else SCH  # causal (copied) width
                        ps = psum.tile([P, SCH], f32, tag="ps")
                        nmm = len(s2_passes) * ND
                        k = 0
                        for lhs_src, rhs_src in s2_passes:
                            for ec in range(ND):
                                nc.tensor.matmul(
                                    ps[:, :w],
                                    lhs_src[:, ec, its],
                                    rhs_src[:, ec, j * SCH : j * SCH + w],
                                    start=(k == 0),
                                    stop=(k == nmm - 1),
                                )
                                k += 1
                        if last:
                            # diag chunk: SBUF copy + causal mask (PSUM must
                            # stay PE-written-only for ACT readers)
                            sc = scs.tile([P, SCH], f32, tag="sc")
                            nc.vector.tensor_copy(sc[:, :wc], ps[:, :wc])
                            nc.vector.tensor_add(
                                sc[:, dcol : dcol + P], sc[:, dcol : dcol + P], cmask
                            )
                            src_t = sc
                        else:
                            src_t = ps
                        nc.vector.reduce_max(
                            out=m4[:, j : j + 1], in_=src_t[:, :wc],
                            axis=mybir.AxisListType.X,
                        )
                        chunks.append((src_t, wc))

                    nm = stats.tile([P, 1], f32, tag="nm")
                    nc.vector.reduce_max(
                        out=nm, in_=m4[:, :nch], axis=mybir.AxisListType.X, negate=True
                    )
                    ex_last = None
                    for j, (sc, wc) in enumerate(chunks):
                        ex = escr.tile([P, SCH], f32, tag="ex")
                        nc.scalar.activation(
                            out=ex[:, :wc],
                            in_=sc[:, :wc],
                            func=mybir.ActivationFunctionType.Exp,
                            bias=nm,
                            scale=1.0,
                            accum_out=lp[:, j : j + 1],
                        )
                        if j == nch - 1:
                            ex_last = ex
                    # diag of exp block: mul by identity then row-reduce
                    dscr = stats.tile([P, P], f32, tag="dscr")
                    nc.vector.tensor_mul(dscr, ex_last[:, dcol : dcol + P], ident)
                    ediag = stats.tile([P, 1], f32, tag="ediag")
                    nc.vector.reduce_sum(
                        out=ediag, in_=dscr, axis=mybir.AxisListType.X
                    )
                    lsum = stats.tile([P, 1], f32, tag="lsum")
                    nc.vector.reduce_sum(
                        out=lsum, in_=lp[:, :nch], axis=mybir.AxisListType.X
                    )
                    rl = stats.tile([P, 1], f32, tag="rl")
                    nc.vector.reciprocal(rl, lsum)
                    datt = stats.tile([P, 1], f32, tag="datt")
                    nc.vector.tensor_mul(datt, ediag, rl)

                    hrow2 = hin.tile([P, D], f32, tag="hrow2")
                    nc.sync.dma_start(out=hrow2, in_=hb_t[:, i, :])
                    ot = outp.tile([P, D], f32, tag="ot")
                    nc.vector.tensor_scalar_mul(ot, hrow2, datt)
                    nc.sync.dma_start(out=out2[hd, its, :], in_=ot)

    nc.compile()
    return nc


_NC_CACHE = {}


def _get_nc(stage1=None, stage2=None):
    key = (stage1 or STAGE1, stage2 or STAGE2)
    if key not in _NC_CACHE:
        _NC_CACHE[key] = build_nc(*key)
    return _NC_CACHE[key]


def _consts():
    cmask = np.triu(np.full((P, P), NEG, np.float32), 1)
    ident = np.eye(P, dtype=np.float32)
    return cmask, ident


def make_in_maps(h, A):
    h = np.ascontiguousarray(h, dtype=np.float32)
    A = np.ascontiguousarray(A, dtype=np.float32)
    cmask, ident = _consts()
    in_maps = []
    for c in range(NCORES):
        b = c // 4
        h0 = 2 * (c % 4)
        in_maps.append({"hb": h[b], "A2": np.ascontiguousarray(A[h0 : h0 + 2]),
                        "cmaskd": cmask, "identd": ident})
    return in_maps


def assemble(results):
    full = np.empty((B, H, T, D), dtype=np.float32)
    for c in range(NCORES):
        b = c // 4
        h0 = 2 * (c % 4)
        o = results[c]["out2"]
        full[b, h0] = o[0]
        full[b, h0 + 1] = o[1]
    return full.reshape(B, T, H * D)


def kernel(h, A):
    nc = _get_nc()
    res = bass_utils.run_bass_kernel_spmd(
        nc, make_in_maps(h, A), core_ids=list(range(NCORES))
    )
    return assemble(res.results)
